# revision 7
# baseline (speedup 1.0000x reference)
"""Causal self-attention (B=4, T=2048, D=1024, H=16) on 8 TRN2 NeuronCores.

Sharding: core c handles batch b = c//2 and head-group g = c%2 (8 heads each).
Each core computes, for its (b, g):
    qkv_loc = x[b] @ w_qkv[:, cols(g)]          (q|k|v local, 512 cols each)
    att     = causal_attention(q, k, v)          (8 heads, hd=64)
    y_part  = att @ w_out[rows(g), :]            ([2048, 1024] partial)
Host sums the two partial outputs per batch.

Dtype plan: QKV projection runs in f32r (1 cycle/row at N=512 moving dim,
bit-identical loads via bitcast). Attention phase and output projection
run in bf16 (halved LDWEIGHTS, 2x/4x DVE modes). PSUM accumulation is
fp32 throughout.

Schedule: phase C iterates query-block-outer (it), head-inner; after each
it-epoch the output projection for those 4 token blocks runs immediately,
so out-DMA streams during attention instead of piling up in a tail.

Softmax uses exp on ScalarE with deferred normalization: the rowsum comes
free from a ones-column appended to V; the reciprocal uses the fast
custom-DVE op and is broadcast across partitions on the Pool engine.
Causal masking is in-place Pool-engine affine_select on diagonal tiles.
Q is kept zero-padded to 128 partitions (QTz) so scores matmuls contract
over K=128 and keep the PE HAM clock gate warm.
"""

import numpy as np

import concourse.bass as bass
import concourse.mybir as mybir
from concourse import bacc, tile
from concourse import bass_utils
from concourse.masks import make_identity

# Problem constants (hardcoded per contest contract)
B = 4
T = 2048
D = 1024
H = 16
HD = 64
H_LOC = 8               # heads per core
CLOC = H_LOC * HD       # 512 local head dims
P = 128
N_CORES = 8

F32 = mybir.dt.float32
F32R = mybir.dt.float32r
BF16 = mybir.dt.bfloat16

MM_MODE = "mixed"  # informational; kernel hardcodes f32r proj + bf16 attention


def _build_kernel_body(nc, tc, x_ap, wqkv_ap, wout_ap, out_ap):
    from contextlib import ExitStack

    Exp = mybir.ActivationFunctionType.Exp
    mult = mybir.AluOpType.mult
    PSUM = "PSUM"

    ctx = ExitStack()

    # ---------------- constants ----------------
    const = ctx.enter_context(tc.tile_pool(name="const", bufs=1))
    ident = const.tile([P, P], F32R)
    scratch = const.tile([P, P], F32, tag="idscratch")
    make_identity(nc, scratch)
    nc.vector.tensor_copy(ident, scratch)

    # ---------------- persistent tiles ----------------
    big = ctx.enter_context(tc.tile_pool(name="big", bufs=1))
    xT = big.tile([P, 8, T], F32R)              # [d%128, d//128, t]    64KB
    QTz = big.tile([P, H_LOC, T], BF16)         # zero-padded Q per head 32KB
    KT = big.tile([P, 4, T], BF16)              # [2-head pack, sub, t] 16KB
    V_aug = big.tile([P, 16, H_LOC, HD + 1], BF16)  # [j%128, jb, h, dd|1]
    AT = big.tile([P, 4, T], BF16)              # attention out (like KT) 16KB
    nc.gpsimd.memset(V_aug[:, :, :, HD], 1.0)   # ones col -> rowsums via AV
    # zero the never-written half of each head's padded Q (once)
    for h2 in range(H_LOC):
        r0 = 64 if h2 % 2 == 0 else 0
        nc.gpsimd.memset(QTz[r0:r0 + 64, h2, :], 0.0)

    xa = x_ap.rearrange("(tb p) d -> tb p d", p=P)            # [16, 128, 1024]
    wall = wqkv_ap.rearrange("(o p) c -> p o c", p=P)         # [128, 8, 1536]

    NTB = T // P

    # ---- phase A: x -> xT (transpose), V-proj software-pipelined ----
    with tc.tile_pool(name="lda", bufs=2) as lda, \
         tc.tile_pool(name="ldw", bufs=1) as ldw, \
         tc.tile_pool(name="psA", bufs=2, space=PSUM) as psA:
        w_sb = ldw.tile([P, 8, 3 * CLOC], F32R)
        nc.sync.dma_start(w_sb, wall.bitcast(F32R))

        def vproj(tb):
            psv = psA.tile([P, CLOC], F32, tag="ps_v")
            for k in range(8):
                nc.tensor.matmul(
                    psv,
                    xT[:, k, tb * P:(tb + 1) * P],
                    w_sb[:, k, 2 * CLOC:3 * CLOC],
                    start=(k == 0),
                    stop=(k == 7),
                )
            nc.vector.tensor_copy(
                V_aug[:, tb, :, 0:HD],
                psv.rearrange("p (h d) -> p h d", h=H_LOC),
            )

        for tb in range(NTB):
            xc = lda.tile([P, D], F32R, tag="xin")
            nc.sync.dma_start(xc, xa[tb].bitcast(F32R))
            for half in range(2):
                pt = psA.tile([P, 512], F32R, tag="ps_t")
                for q in range(4):
                    db = half * 4 + q
                    nc.tensor.transpose(
                        pt[:, q * P:(q + 1) * P],
                        xc[:, db * P:(db + 1) * P],
                        ident,
                    )
                nc.vector.tensor_copy(
                    xT[:, half * 4:(half + 1) * 4, tb * P:(tb + 1) * P],
                    pt.rearrange("p (a b) -> p a b", a=4),
                )
            if tb >= 1:
                vproj(tb - 1)
        vproj(NTB - 1)

        # ---- phase B: Q^T / K^T proj: psum[c_block 128, t 512] ----
        with tc.tile_pool(name="psB", bufs=2, space=PSUM) as psB:
            for cb in range(8):
                sub = cb % 4
                for it in range(4):
                    ps = psB.tile([P, 512], F32, tag="ps_qkv")
                    for k in range(8):
                        nc.tensor.matmul(
                            ps,
                            w_sb[:, k, cb * P:(cb + 1) * P],
                            xT[:, k, it * 512:(it + 1) * 512],
                            start=(k == 0),
                            stop=(k == 7),
                        )
                    cols = slice(it * 512, (it + 1) * 512)
                    if cb < 4:
                        # Q: split the two packed heads into padded QTz slots
                        nc.vector.tensor_copy(
                            QTz[0:64, 2 * sub, cols], ps[0:64, :]
                        )
                        nc.vector.tensor_copy(
                            QTz[64:128, 2 * sub + 1, cols], ps[64:128, :]
                        )
                    else:
                        nc.vector.tensor_copy(KT[:, sub, cols], ps)

    # preload + cast w_out during attention so phase D starts immediately
    ldo = ctx.enter_context(tc.tile_pool(name="ldo", bufs=1))
    wo_st = ldo.tile([P, 4, D], F32, tag="wo_st")
    nc.sync.dma_start(wo_st, wout_ap.rearrange("(o p) n -> p o n", p=P))
    wo_sb = ldo.tile([P, 4, D], BF16, tag="wo_sb")
    nc.vector.tensor_copy(wo_sb, wo_st)

    oa = out_ap.rearrange("(tb p) d -> tb p d", p=P)

    # ---------------- phase C + D interleaved ----------------
    with tc.tile_pool(name="att", bufs=4) as att_pool, \
         tc.tile_pool(name="sm", bufs=2) as sm_pool, \
         tc.tile_pool(name="ypool", bufs=3) as ypool, \
         tc.tile_pool(name="psS", bufs=2, space=PSUM) as psS, \
         tc.tile_pool(name="psO", bufs=2, space=PSUM) as psO, \
         tc.tile_pool(name="psD", bufs=2, space=PSUM) as psD:
        for it in range(4):
            i0 = it * 512
            njb = 4 * (it + 1)
            for h in range(H_LOC):
                row0 = (h % 2) * 64
                sub = h // 2
                po = psO.tile([P, 512], F32, tag="ps_o")
                for jb2 in range(njb // 2):
                    ps = psS.tile([P, 1024], F32, tag="ps_s")
                    for u in range(2):
                        jb = 2 * jb2 + u
                        nc.tensor.matmul(
                            ps[:, u * 512:(u + 1) * 512],
                            KT[:, sub, jb * P:(jb + 1) * P],
                            QTz[:, h, i0:i0 + 512],
                            start=True,
                            stop=True,
                        )
                    es = att_pool.tile([P, 1024], BF16, tag="es")
                    nc.scalar.activation(es, ps, Exp, scale=0.125)
                    for u in range(2):
                        jb = 2 * jb2 + u
                        off = jb * P - i0
                        if off >= 0:  # diagonal tile: zero out j > i
                            nc.gpsimd.affine_select(
                                out=es[:, u * 512:(u + 1) * 512],
                                in_=es[:, u * 512:(u + 1) * 512],
                                compare_op=mybir.AluOpType.is_ge,
                                fill=0.0,
                                base=-off,
                                channel_multiplier=-1,
                                pattern=[[1, 512]],
                            )
                    for u in range(2):
                        jb = 2 * jb2 + u
                        nc.tensor.matmul(
                            po[0:HD + 1, :],
                            V_aug[:, jb, h, :],
                            es[:, u * 512:(u + 1) * 512],
                            start=(jb == 0),
                            stop=(jb == njb - 1),
                        )
                # deferred softmax normalization
                rr = sm_pool.tile([1, 512], F32, tag="rr")
                nc.vector.tensor_copy(rr, po[HD:HD + 1, :])
                rf = sm_pool.tile([1, 512], F32, tag="rf")
                nc.vector.reciprocal_approx_fast(out=rf, in_=rr)
                rb = sm_pool.tile([64, 512], F32, tag="rb")
                nc.gpsimd.partition_broadcast(rb, rf)
                nc.vector.tensor_tensor(
                    AT[row0:row0 + 64, sub, i0:i0 + 512],
                    po[0:64, :],
                    rb,
                    mult,
                )
            # ---- phase D chunk for this epoch's 4 token blocks ----
            for tb in range(4 * it, 4 * it + 4):
                for nt in range(2):
                    py = psD.tile([P, 512], F32, tag="ps_y")
                    for k in range(4):
                        nc.tensor.matmul(
                            py,
                            AT[:, k, tb * P:(tb + 1) * P],
                            wo_sb[:, k, nt * 512:(nt + 1) * 512],
                            start=(k == 0),
                            stop=(k == 3),
                        )
                    ysb = ypool.tile([P, 512], F32, tag="ysb")
                    nc.vector.tensor_copy(ysb, py)
                    nc.sync.dma_start(oa[tb, :, nt * 512:(nt + 1) * 512], ysb)

    ctx.close()


_CACHE = {}


def _get_nc(mode=None):
    key = "mixed"
    if key in _CACHE:
        return _CACHE[key]
    nc = bacc.Bacc(
        "TRN2",
        target_bir_lowering=False,
        debug=False,
        enable_asserts=False,
        num_devices=N_CORES,
    )
    x_d = nc.dram_tensor("x", [T, D], F32, kind="ExternalInput")
    wqkv_d = nc.dram_tensor("w_qkv", [D, 3 * CLOC], F32, kind="ExternalInput")
    wout_d = nc.dram_tensor("w_out", [CLOC, D], F32, kind="ExternalInput")
    out_d = nc.dram_tensor("out", [T, D], F32, kind="ExternalOutput")
    with tile.TileContext(nc) as tc:
        _build_kernel_body(
            nc, tc, x_d.ap(), wqkv_d.ap(), wout_d.ap(), out_d.ap()
        )
    nc.compile()
    _CACHE[key] = nc
    return nc


def _make_in_maps(x, w_qkv, w_out):
    x = np.ascontiguousarray(np.asarray(x, dtype=np.float32))
    w_qkv = np.ascontiguousarray(np.asarray(w_qkv, dtype=np.float32))
    w_out = np.ascontiguousarray(np.asarray(w_out, dtype=np.float32))
    in_maps = []
    for c in range(N_CORES):
        b, g = divmod(c, 2)
        c0 = g * CLOC
        wloc = np.concatenate(
            [
                w_qkv[:, c0:c0 + CLOC],
                w_qkv[:, D + c0:D + c0 + CLOC],
                w_qkv[:, 2 * D + c0:2 * D + c0 + CLOC],
            ],
            axis=1,
        )
        in_maps.append({
            "x": np.ascontiguousarray(x[b]),
            "w_qkv": np.ascontiguousarray(wloc),
            "w_out": np.ascontiguousarray(w_out[c0:c0 + CLOC]),
        })
    return in_maps


def run(x, w_qkv, w_out, trace=False, mode=None):
    nc = _get_nc(mode)
    in_maps = _make_in_maps(x, w_qkv, w_out)
    res = bass_utils.run_bass_kernel_spmd(
        nc, in_maps, core_ids=list(range(N_CORES)), trace=trace
    )
    y = np.empty((B, T, D), dtype=np.float32)
    for b in range(B):
        y[b] = res.results[2 * b]["out"] + res.results[2 * b + 1]["out"]
    return y, res


def kernel(x, w_qkv, w_out):
    y, _ = run(x, w_qkv, w_out, trace=False)
    return y


# revision 11
# speedup vs baseline: 1.0633x; 1.0633x over previous
"""Causal self-attention (B=4, T=2048, D=1024, H=16) on 8 TRN2 NeuronCores.

Sharding: core c handles batch b = c//2 and head-group g = c%2 (8 heads each).
Each core computes, for its (b, g):
    qkv_loc = x[b] @ w_qkv[:, cols(g)]          (q|k|v local, 512 cols each)
    att     = causal_attention(q, k, v)          (8 heads, hd=64)
    y_part  = att @ w_out[rows(g), :]            ([2048, 1024] partial)
Host sums the two partial outputs per batch.

Dtype plan: projections (QKV, out) run in f32r (1 cycle/row for N>=512
moving dim, bit-identical loads via bitcast). Attention inner phase runs
in bf16 (halved LDWEIGHTS, 2x/4x DVE modes). PSUM accumulation is fp32
throughout. Softmax uses exp on ScalarE with deferred normalization: the
rowsum comes free from a ones-column appended to V; the reciprocal is
computed with the fast custom-DVE op and broadcast across partitions on
the (otherwise idle) Pool engine. Causal masking is done in-place on the
exp'd scores with Pool-engine affine_select (only diagonal tiles).
"""

import numpy as np

import concourse.bass as bass
import concourse.mybir as mybir
from concourse import bacc, tile
from concourse import bass_utils
from concourse.masks import make_identity

# Problem constants (hardcoded per contest contract)
B = 4
T = 2048
D = 1024
H = 16
HD = 64
H_LOC = 8               # heads per core
CLOC = H_LOC * HD       # 512 local head dims
P = 128
N_CORES = 8

F32 = mybir.dt.float32
F32R = mybir.dt.float32r
BF16 = mybir.dt.bfloat16

MM_MODE = "mixed"  # informational; kernel hardcodes f32r proj + bf16 attention


def _build_kernel_body(nc, tc, x_ap, wqkv_ap, wout_ap, out_ap):
    from contextlib import ExitStack

    Exp = mybir.ActivationFunctionType.Exp
    mult = mybir.AluOpType.mult
    PSUM = "PSUM"

    ctx = ExitStack()

    # ---------------- constants ----------------
    const = ctx.enter_context(tc.tile_pool(name="const", bufs=1))
    ident = const.tile([P, P], F32R)
    scratch = const.tile([P, P], F32, tag="idscratch")
    make_identity(nc, scratch)
    nc.vector.tensor_copy(ident, scratch)

    # causal mask helper: wm[p, f] = 1.0 iff f - p - 384 >= 0
    wm = const.tile([P, 896], BF16, tag="wm")
    nc.gpsimd.memset(wm, 1.0)
    nc.gpsimd.affine_select(
        out=wm,
        in_=wm,
        compare_op=mybir.AluOpType.is_ge,
        fill=0.0,
        base=-384,
        channel_multiplier=-1,
        pattern=[[1, 896]],
    )

    # ---------------- persistent tiles ----------------
    big = ctx.enter_context(tc.tile_pool(name="big", bufs=1))
    xT = big.tile([P, 8, T], F32R)              # [d%128, d//128, t]    64KB
    QT = big.tile([P, 4, T], BF16)              # [2-head pack, sub, t] 16KB
    KT = big.tile([P, 4, T], BF16)              # 16KB
    V_aug = big.tile([P, 16, H_LOC, HD + 1], BF16)  # [j%128, jb, h, dd|1] 16.25KB
    AT = big.tile([P, 4, T], F32R)              # attention out (like QT) 32KB
    nc.gpsimd.memset(V_aug[:, :, :, HD], 1.0)   # ones col -> rowsums via AV

    xa = x_ap.rearrange("(tb p) d -> tb p d", p=P)            # [16, 128, 1024]
    wall = wqkv_ap.rearrange("(o p) c -> p o c", p=P)         # [128, 8, 1536]

    NTB = T // P

    # ---- phase A: x -> xT (transpose), V-proj software-pipelined ----
    with tc.tile_pool(name="lda", bufs=3) as lda, \
         tc.tile_pool(name="ldw", bufs=1) as ldw, \
         tc.tile_pool(name="psA", bufs=2, space=PSUM) as psA:
        w_sb = ldw.tile([P, 8, 3 * CLOC], F32R)
        nc.sync.dma_start(w_sb, wall.bitcast(F32R))

        def vproj(tb):
            psv = psA.tile([P, CLOC], F32, tag="ps_v")
            for k in range(8):
                nc.tensor.matmul(
                    psv,
                    xT[:, k, tb * P:(tb + 1) * P],
                    w_sb[:, k, 2 * CLOC:3 * CLOC],
                    start=(k == 0),
                    stop=(k == 7),
                )
            nc.vector.tensor_copy(
                V_aug[:, tb, :, 0:HD],
                psv.rearrange("p (h d) -> p h d", h=H_LOC),
            )

        for tb in range(NTB):
            xc = lda.tile([P, D], F32R, tag="xin")
            nc.sync.dma_start(xc, xa[tb].bitcast(F32R))
            for half in range(2):
                pt = psA.tile([P, 512], F32R, tag="ps_t")
                for q in range(4):
                    db = half * 4 + q
                    nc.tensor.transpose(
                        pt[:, q * P:(q + 1) * P],
                        xc[:, db * P:(db + 1) * P],
                        ident,
                    )
                nc.vector.tensor_copy(
                    xT[:, half * 4:(half + 1) * 4, tb * P:(tb + 1) * P],
                    pt.rearrange("p (a b) -> p a b", a=4),
                )
            if tb >= 1:
                vproj(tb - 1)
        vproj(NTB - 1)

        # ---- phase B: Q^T / K^T proj: psum[c_block 128, t 512] ----
        with tc.tile_pool(name="psB", bufs=2, space=PSUM) as psB:
            for cb in range(8):
                dest = QT if cb < 4 else KT
                sub = cb % 4
                for it in range(4):
                    ps = psB.tile([P, 512], F32, tag="ps_qkv")
                    for k in range(8):
                        nc.tensor.matmul(
                            ps,
                            w_sb[:, k, cb * P:(cb + 1) * P],
                            xT[:, k, it * 512:(it + 1) * 512],
                            start=(k == 0),
                            stop=(k == 7),
                        )
                    nc.vector.tensor_copy(
                        dest[:, sub, it * 512:(it + 1) * 512], ps
                    )

    # preload w_out during attention so phase D starts immediately
    ldo = ctx.enter_context(tc.tile_pool(name="ldo", bufs=1))
    wo_sb = ldo.tile([P, 4, D], F32R)
    nc.sync.dma_start(wo_sb, wout_ap.rearrange("(o p) n -> p o n", p=P).bitcast(F32R))

    # ---------------- phase C: causal attention ----------------
    # Scores matmuls contract over K=128 partitions (2 packed heads; the
    # moving Q operand has the other parity's 64 rows zeroed) to keep the
    # PE HAM clock gate warm.
    with tc.tile_pool(name="att", bufs=4) as att_pool, \
         tc.tile_pool(name="sm", bufs=2) as sm_pool, \
         tc.tile_pool(name="qp", bufs=1) as qp_pool, \
         tc.tile_pool(name="psS", bufs=3, space=PSUM) as psS, \
         tc.tile_pool(name="psO", bufs=2, space=PSUM) as psO:
        Qp0 = qp_pool.tile([P, T], BF16)        # padded Q scratch, even heads
        Qp1 = qp_pool.tile([P, T], BF16, tag="qp1")
        nc.gpsimd.memset(Qp0[64:128, :], 0.0)
        nc.gpsimd.memset(Qp1[0:64, :], 0.0)
        for h in range(H_LOC):
            row0 = (h % 2) * 64
            sub = h // 2
            Qph = Qp0 if h % 2 == 0 else Qp1
            nc.vector.tensor_copy(
                Qph[row0:row0 + 64, :], QT[row0:row0 + 64, sub, :]
            )
            for it in range(4):
                i0 = it * 512
                njb = 4 * (it + 1)
                po = psO.tile([P, 512], F32, tag="ps_o")
                for jb2 in range(njb // 2):
                    ps = psS.tile([P, 1024], F32, tag="ps_s")
                    for u in range(2):
                        jb = 2 * jb2 + u
                        nc.tensor.matmul(
                            ps[:, u * 512:(u + 1) * 512],
                            KT[:, sub, jb * P:(jb + 1) * P],
                            Qph[:, i0:i0 + 512],
                            start=True,
                            stop=True,
                        )
                    es = att_pool.tile([P, 1024], BF16, tag="es")
                    nc.scalar.activation(es, ps, Exp, scale=0.125)
                    for u in range(2):
                        jb = 2 * jb2 + u
                        off = jb * P - i0
                        if off >= 0:  # diagonal tile: zero out j > i
                            s = 384 - off
                            nc.vector.tensor_tensor(
                                es[:, u * 512:(u + 1) * 512],
                                es[:, u * 512:(u + 1) * 512],
                                wm[:, s:s + 512],
                                mult,
                            )
                    for u in range(2):
                        jb = 2 * jb2 + u
                        nc.tensor.matmul(
                            po[0:HD + 1, :],
                            V_aug[:, jb, h, :],
                            es[:, u * 512:(u + 1) * 512],
                            start=(jb == 0),
                            stop=(jb == njb - 1),
                        )
                # deferred softmax normalization
                rr = sm_pool.tile([1, 512], F32, tag="rr")
                nc.vector.tensor_copy(rr, po[HD:HD + 1, :])
                rf = sm_pool.tile([1, 512], F32, tag="rf")
                nc.vector.reciprocal_approx_fast(out=rf, in_=rr)
                rb = sm_pool.tile([64, 512], F32, tag="rb")
                nc.gpsimd.partition_broadcast(rb, rf)
                nc.vector.tensor_tensor(
                    AT[row0:row0 + 64, sub, i0:i0 + 512],
                    po[0:64, :],
                    rb,
                    mult,
                )

    # ---------------- phase D: output projection ----------------
    oa = out_ap.rearrange("(tb p) d -> tb p d", p=P)
    with tc.tile_pool(name="ypool", bufs=3) as ypool, \
         tc.tile_pool(name="psD", bufs=4, space=PSUM) as psD:
        for tb in range(T // P):
            for nt in range(2):
                py = psD.tile([P, 512], F32, tag="ps_y")
                for k in range(4):
                    nc.tensor.matmul(
                        py,
                        AT[:, k, tb * P:(tb + 1) * P],
                        wo_sb[:, k, nt * 512:(nt + 1) * 512],
                        start=(k == 0),
                        stop=(k == 3),
                    )
                ysb = ypool.tile([P, 512], F32, tag="ysb")
                nc.vector.tensor_copy(ysb, py)
                nc.sync.dma_start(oa[tb, :, nt * 512:(nt + 1) * 512], ysb)

    ctx.close()


_CACHE = {}


def _get_nc(mode=None):
    key = "mixed"
    if key in _CACHE:
        return _CACHE[key]
    nc = bacc.Bacc(
        "TRN2",
        target_bir_lowering=False,
        debug=False,
        enable_asserts=False,
        num_devices=N_CORES,
    )
    x_d = nc.dram_tensor("x", [T, D], F32, kind="ExternalInput")
    wqkv_d = nc.dram_tensor("w_qkv", [D, 3 * CLOC], F32, kind="ExternalInput")
    wout_d = nc.dram_tensor("w_out", [CLOC, D], F32, kind="ExternalInput")
    out_d = nc.dram_tensor("out", [T, D], F32, kind="ExternalOutput")
    with tile.TileContext(nc) as tc:
        _build_kernel_body(
            nc, tc, x_d.ap(), wqkv_d.ap(), wout_d.ap(), out_d.ap()
        )
    nc.compile()
    _CACHE[key] = nc
    return nc


def _make_in_maps(x, w_qkv, w_out):
    x = np.ascontiguousarray(np.asarray(x, dtype=np.float32))
    w_qkv = np.ascontiguousarray(np.asarray(w_qkv, dtype=np.float32))
    w_out = np.ascontiguousarray(np.asarray(w_out, dtype=np.float32))
    in_maps = []
    for c in range(N_CORES):
        b, g = divmod(c, 2)
        c0 = g * CLOC
        wloc = np.concatenate(
            [
                w_qkv[:, c0:c0 + CLOC],
                w_qkv[:, D + c0:D + c0 + CLOC],
                w_qkv[:, 2 * D + c0:2 * D + c0 + CLOC],
            ],
            axis=1,
        )
        in_maps.append({
            "x": np.ascontiguousarray(x[b]),
            "w_qkv": np.ascontiguousarray(wloc),
            "w_out": np.ascontiguousarray(w_out[c0:c0 + CLOC]),
        })
    return in_maps


def run(x, w_qkv, w_out, trace=False, mode=None):
    nc = _get_nc(mode)
    in_maps = _make_in_maps(x, w_qkv, w_out)
    res = bass_utils.run_bass_kernel_spmd(
        nc, in_maps, core_ids=list(range(N_CORES)), trace=trace
    )
    y = np.empty((B, T, D), dtype=np.float32)
    for b in range(B):
        y[b] = res.results[2 * b]["out"] + res.results[2 * b + 1]["out"]
    return y, res


def kernel(x, w_qkv, w_out):
    y, _ = run(x, w_qkv, w_out, trace=False)
    return y


# revision 17
# speedup vs baseline: 1.0780x; 1.0138x over previous
"""Causal self-attention (B=4, T=2048, D=1024, H=16) on 8 TRN2 NeuronCores.

Sharding: core c handles batch b = c//2 and head-group g = c%2 (8 heads each).
Each core computes, for its (b, g):
    qkv_loc = x[b] @ w_qkv[:, cols(g)]          (q|k|v local, 512 cols each)
    att     = causal_attention(q, k, v)          (8 heads, hd=64)
    y_part  = att @ w_out[rows(g), :]            ([2048, 1024] partial)
Host sums the two partial outputs per batch.

Dtype plan: projections (QKV, out) run in f32r (1 cycle/row for N>=512
moving dim, bit-identical loads via bitcast). Attention inner phase runs
in bf16 (halved LDWEIGHTS, 2x/4x DVE modes). PSUM accumulation is fp32
throughout. Softmax uses exp on ScalarE with deferred normalization: the
rowsum comes free from a ones-column appended to V; the reciprocal is
computed with the fast custom-DVE op and broadcast across partitions on
the (otherwise idle) Pool engine. Causal masking is done in-place on the
exp'd scores with Pool-engine affine_select (only diagonal tiles).
"""

import numpy as np

import concourse.bass as bass
import concourse.mybir as mybir
from concourse import bacc, tile
from concourse import bass_utils
from concourse.masks import make_identity

# Problem constants (hardcoded per contest contract)
B = 4
T = 2048
D = 1024
H = 16
HD = 64
H_LOC = 8               # heads per core
CLOC = H_LOC * HD       # 512 local head dims
P = 128
N_CORES = 8

F32 = mybir.dt.float32
F32R = mybir.dt.float32r
BF16 = mybir.dt.bfloat16

MM_MODE = "mixed"  # informational; kernel hardcodes f32r proj + bf16 attention


def _build_kernel_body(nc, tc, x_ap, wqkv_ap, wout_ap, out_ap):
    from contextlib import ExitStack

    Exp = mybir.ActivationFunctionType.Exp
    mult = mybir.AluOpType.mult
    PSUM = "PSUM"

    ctx = ExitStack()

    # ---------------- constants ----------------
    const = ctx.enter_context(tc.tile_pool(name="const", bufs=1))
    ident = const.tile([P, P], F32R)
    scratch = const.tile([P, P], F32, tag="idscratch")
    make_identity(nc, scratch)
    nc.vector.tensor_copy(ident, scratch)

    # causal mask helper: wm[p, f] = 1.0 iff f - p - 384 >= 0
    wm = const.tile([P, 896], BF16, tag="wm")
    nc.gpsimd.memset(wm, 1.0)
    nc.gpsimd.affine_select(
        out=wm,
        in_=wm,
        compare_op=mybir.AluOpType.is_ge,
        fill=0.0,
        base=-384,
        channel_multiplier=-1,
        pattern=[[1, 896]],
    )

    # ---------------- persistent tiles ----------------
    big = ctx.enter_context(tc.tile_pool(name="big", bufs=1))
    xT = big.tile([P, 8, T], F32R)              # [d%128, d//128, t]    64KB
    QT = big.tile([P, 4, T], BF16)              # [2-head pack, sub, t] 16KB
    KT = big.tile([P, 4, T], BF16)              # 16KB
    V_aug = big.tile([P, 16, H_LOC, HD + 1], BF16)  # [j%128, jb, h, dd|1] 16.25KB
    AT = big.tile([P, 4, T], F32R)              # attention out (like QT) 32KB
    nc.gpsimd.memset(V_aug[:, :, :, HD], 1.0)   # ones col -> rowsums via AV

    xa = x_ap.rearrange("(tb p) d -> tb p d", p=P)            # [16, 128, 1024]
    wall = wqkv_ap.rearrange("(o p) c -> p o c", p=P)         # [128, 8, 1536]

    NTB = T // P

    # ---- phase A: x -> xT (transpose), V-proj software-pipelined ----
    with tc.tile_pool(name="lda", bufs=3) as lda, \
         tc.tile_pool(name="ldw", bufs=1) as ldw, \
         tc.tile_pool(name="psA", bufs=2, space=PSUM) as psA:
        # first two x tiles go out ahead of the (large) weight DMAs so the
        # transposes can start immediately; V weights load before QK.
        pre = {}
        for tb in range(2):
            xc = lda.tile([P, D], F32R, tag="xin")
            nc.sync.dma_start(xc, xa[tb].bitcast(F32R))
            pre[tb] = xc
        wv_sb = ldw.tile([P, 8, CLOC], F32R, tag="wv")
        nc.sync.dma_start(wv_sb, wall[:, :, 2 * CLOC:3 * CLOC].bitcast(F32R))
        w_sb = ldw.tile([P, 8, 2 * CLOC], F32R, tag="wqk")
        nc.sync.dma_start(w_sb, wall[:, :, 0:2 * CLOC].bitcast(F32R))

        def vproj(tb):
            psv = psA.tile([P, CLOC], F32, tag="ps_v")
            for k in range(8):
                nc.tensor.matmul(
                    psv,
                    xT[:, k, tb * P:(tb + 1) * P],
                    wv_sb[:, k, :],
                    start=(k == 0),
                    stop=(k == 7),
                )
            nc.vector.tensor_copy(
                V_aug[:, tb, :, 0:HD],
                psv.rearrange("p (h d) -> p h d", h=H_LOC),
            )

        for tb in range(NTB):
            if tb in pre:
                xc = pre.pop(tb)
            else:
                xc = lda.tile([P, D], F32R, tag="xin")
                nc.sync.dma_start(xc, xa[tb].bitcast(F32R))
            for half in range(2):
                pt = psA.tile([P, 512], F32R, tag="ps_t")
                for q in range(4):
                    db = half * 4 + q
                    nc.tensor.transpose(
                        pt[:, q * P:(q + 1) * P],
                        xc[:, db * P:(db + 1) * P],
                        ident,
                    )
                nc.vector.tensor_copy(
                    xT[:, half * 4:(half + 1) * 4, tb * P:(tb + 1) * P],
                    pt.rearrange("p (a b) -> p a b", a=4),
                )
            if tb >= 1:
                vproj(tb - 1)
        vproj(NTB - 1)

        # ---- phase B: Q^T / K^T proj: psum[c_block 128, t 512] ----
        with tc.tile_pool(name="psB", bufs=2, space=PSUM) as psB:
            for cb in range(8):
                dest = QT if cb < 4 else KT
                sub = cb % 4
                for it in range(4):
                    ps = psB.tile([P, 512], F32, tag="ps_qkv")
                    for k in range(8):
                        nc.tensor.matmul(
                            ps,
                            w_sb[:, k, cb * P:(cb + 1) * P],
                            xT[:, k, it * 512:(it + 1) * 512],
                            start=(k == 0),
                            stop=(k == 7),
                        )
                    nc.vector.tensor_copy(
                        dest[:, sub, it * 512:(it + 1) * 512], ps
                    )

    # preload w_out during attention so phase D starts immediately
    ldo = ctx.enter_context(tc.tile_pool(name="ldo", bufs=1))
    wo_sb = ldo.tile([P, 4, D], F32R)
    nc.sync.dma_start(wo_sb, wout_ap.rearrange("(o p) n -> p o n", p=P).bitcast(F32R))

    # ---------------- phase C: causal attention ----------------
    # Scores matmuls contract over K=128 partitions (2 packed heads; the
    # moving Q operand has the other parity's 64 rows zeroed) to keep the
    # PE HAM clock gate warm.
    with tc.tile_pool(name="att", bufs=4) as att_pool, \
         tc.tile_pool(name="sm", bufs=2) as sm_pool, \
         tc.tile_pool(name="qp", bufs=1) as qp_pool, \
         tc.tile_pool(name="psS", bufs=3, space=PSUM) as psS, \
         tc.tile_pool(name="psO", bufs=2, space=PSUM) as psO:
        # ring of 4 padded-Q scratches so head h+2's staging copy never
        # waits on head h's scores (WAR) and Act never starves at head
        # boundaries
        qp_a = qp_pool.tile([P, T], BF16, tag="qp0")
        qp_b = qp_pool.tile([P, T], BF16, tag="qp1")
        qp_c = qp_pool.tile([P, T], BF16, tag="qp2")
        qp_d = qp_pool.tile([P, T], BF16, tag="qp3")
        Qps = [qp_a, qp_b, qp_c, qp_d]
        for i in range(4):
            z0 = 64 if i % 2 == 0 else 0
            nc.gpsimd.memset(Qps[i][z0:z0 + 64, :], 0.0)

        def qp_copy(h):
            row0 = (h % 2) * 64
            nc.vector.tensor_copy(
                Qps[h % 4][row0:row0 + 64, :], QT[row0:row0 + 64, h // 2, :]
            )

        qp_copy(0)
        qp_copy(1)
        for h in range(H_LOC):
            row0 = (h % 2) * 64
            sub = h // 2
            Qph = Qps[h % 4]
            if h + 2 < H_LOC:
                qp_copy(h + 2)
            for it in range(4):
                i0 = it * 512
                njb = 4 * (it + 1)
                po = psO.tile([P, 512], F32, tag="ps_o")
                for jb2 in range(njb // 2):
                    ps = psS.tile([P, 1024], F32, tag="ps_s")
                    for u in range(2):
                        jb = 2 * jb2 + u
                        nc.tensor.matmul(
                            ps[:, u * 512:(u + 1) * 512],
                            KT[:, sub, jb * P:(jb + 1) * P],
                            Qph[:, i0:i0 + 512],
                            start=True,
                            stop=True,
                        )
                    es = att_pool.tile([P, 1024], BF16, tag="es")
                    nc.scalar.activation(es, ps, Exp, scale=0.125)
                    for u in range(2):
                        jb = 2 * jb2 + u
                        off = jb * P - i0
                        if off >= 0:  # diagonal tile: zero out j > i
                            s = 384 - off
                            nc.vector.tensor_tensor(
                                es[:, u * 512:(u + 1) * 512],
                                es[:, u * 512:(u + 1) * 512],
                                wm[:, s:s + 512],
                                mult,
                            )
                    for u in range(2):
                        jb = 2 * jb2 + u
                        nc.tensor.matmul(
                            po[0:HD + 1, :],
                            V_aug[:, jb, h, :],
                            es[:, u * 512:(u + 1) * 512],
                            start=(jb == 0),
                            stop=(jb == njb - 1),
                        )
                # deferred softmax normalization
                rr = sm_pool.tile([1, 512], F32, tag="rr")
                nc.vector.tensor_copy(rr, po[HD:HD + 1, :])
                rf = sm_pool.tile([1, 512], F32, tag="rf")
                nc.vector.reciprocal_approx_fast(out=rf, in_=rr)
                rb = sm_pool.tile([64, 512], F32, tag="rb")
                nc.gpsimd.partition_broadcast(rb, rf)
                nc.vector.tensor_tensor(
                    AT[row0:row0 + 64, sub, i0:i0 + 512],
                    po[0:64, :],
                    rb,
                    mult,
                )

    # ---------------- phase D: output projection ----------------
    oa = out_ap.rearrange("(tb p) d -> tb p d", p=P)
    with tc.tile_pool(name="ypool", bufs=3) as ypool, \
         tc.tile_pool(name="psD", bufs=4, space=PSUM) as psD:
        for tb in range(T // P):
            for nt in range(2):
                py = psD.tile([P, 512], F32, tag="ps_y")
                for k in range(4):
                    nc.tensor.matmul(
                        py,
                        AT[:, k, tb * P:(tb + 1) * P],
                        wo_sb[:, k, nt * 512:(nt + 1) * 512],
                        start=(k == 0),
                        stop=(k == 3),
                    )
                ysb = ypool.tile([P, 512], BF16, tag="ysb")
                nc.vector.tensor_copy(ysb, py)
                nc.sync.dma_start(oa[tb, :, nt * 512:(nt + 1) * 512], ysb)

    ctx.close()


_CACHE = {}


def _get_nc(mode=None):
    key = "mixed"
    if key in _CACHE:
        return _CACHE[key]
    nc = bacc.Bacc(
        "TRN2",
        target_bir_lowering=False,
        debug=False,
        enable_asserts=False,
        num_devices=N_CORES,
    )
    x_d = nc.dram_tensor("x", [T, D], F32, kind="ExternalInput")
    wqkv_d = nc.dram_tensor("w_qkv", [D, 3 * CLOC], F32, kind="ExternalInput")
    wout_d = nc.dram_tensor("w_out", [CLOC, D], F32, kind="ExternalInput")
    out_d = nc.dram_tensor("out", [T, D], BF16, kind="ExternalOutput")
    with tile.TileContext(nc) as tc:
        _build_kernel_body(
            nc, tc, x_d.ap(), wqkv_d.ap(), wout_d.ap(), out_d.ap()
        )
    nc.compile()
    _CACHE[key] = nc
    return nc


def _make_in_maps(x, w_qkv, w_out):
    x = np.ascontiguousarray(np.asarray(x, dtype=np.float32))
    w_qkv = np.ascontiguousarray(np.asarray(w_qkv, dtype=np.float32))
    w_out = np.ascontiguousarray(np.asarray(w_out, dtype=np.float32))
    in_maps = []
    for c in range(N_CORES):
        b, g = divmod(c, 2)
        c0 = g * CLOC
        wloc = np.concatenate(
            [
                w_qkv[:, c0:c0 + CLOC],
                w_qkv[:, D + c0:D + c0 + CLOC],
                w_qkv[:, 2 * D + c0:2 * D + c0 + CLOC],
            ],
            axis=1,
        )
        in_maps.append({
            "x": np.ascontiguousarray(x[b]),
            "w_qkv": np.ascontiguousarray(wloc),
            "w_out": np.ascontiguousarray(w_out[c0:c0 + CLOC]),
        })
    return in_maps


def run(x, w_qkv, w_out, trace=False, mode=None):
    nc = _get_nc(mode)
    in_maps = _make_in_maps(x, w_qkv, w_out)
    res = bass_utils.run_bass_kernel_spmd(
        nc, in_maps, core_ids=list(range(N_CORES)), trace=trace
    )
    y = np.empty((B, T, D), dtype=np.float32)
    for b in range(B):
        y[b] = (
            np.asarray(res.results[2 * b]["out"], dtype=np.float32)
            + np.asarray(res.results[2 * b + 1]["out"], dtype=np.float32)
        )
    return y, res


def kernel(x, w_qkv, w_out):
    y, _ = run(x, w_qkv, w_out, trace=False)
    return y


# revision 19
# speedup vs baseline: 1.0851x; 1.0066x over previous
"""Causal self-attention (B=4, T=2048, D=1024, H=16) on 8 TRN2 NeuronCores.

Sharding: core c handles batch b = c//2 and head-group g = c%2 (8 heads each).
Each core computes, for its (b, g):
    qkv_loc = x[b] @ w_qkv[:, cols(g)]          (q|k|v local, 512 cols each)
    att     = causal_attention(q, k, v)          (8 heads, hd=64)
    y_part  = att @ w_out[rows(g), :]            ([2048, 1024] partial)
Host sums the two partial outputs per batch (bf16 partials, fp32 sum).

Dtype plan: QKV projection runs in f32r (1 cycle/row at N=512 moving dim,
bit-identical loads via bitcast); attention and the output projection run
in bf16 (halved LDWEIGHTS, 2x/4x DVE modes); PSUM accumulates fp32.

Phase C emits a flat stream of (it, h, jb2) score tiles with a one-tile
software lag between the scores matmuls and the attention-weighted V
matmuls, so the PE never drains while exp (ScalarE) and the causal-mask
multiply (DVE) of the newest tile are in flight. Iteration is
query-block-outer, so each token block's output projection (phase D
piece) is ready one it-epoch later and is interleaved at softmax
normalization points — the out-DMA streams during attention instead of
piling up in a tail.

Softmax is max-free exp with deferred normalization: rowsums come free
from a ones-column appended to V; the reciprocal uses the fast custom-DVE
op and is broadcast across partitions by the Pool engine. Q lives
zero-padded to 128 partitions (QTz, built directly by phase B's PSUM
evacuations) so scores matmuls contract over K=128 and keep the PE HAM
clock gate warm.
"""

import numpy as np

import concourse.bass as bass
import concourse.mybir as mybir
from concourse import bacc, tile
from concourse import bass_utils
from concourse.masks import make_identity

# Problem constants (hardcoded per contest contract)
B = 4
T = 2048
D = 1024
H = 16
HD = 64
H_LOC = 8               # heads per core
CLOC = H_LOC * HD       # 512 local head dims
P = 128
N_CORES = 8

F32 = mybir.dt.float32
F32R = mybir.dt.float32r
BF16 = mybir.dt.bfloat16

MM_MODE = "mixed"  # informational; kernel hardcodes f32r proj + bf16 attention


def _build_kernel_body(nc, tc, x_ap, wqkv_ap, wout_ap, out_ap):
    from contextlib import ExitStack

    Exp = mybir.ActivationFunctionType.Exp
    mult = mybir.AluOpType.mult
    PSUM = "PSUM"

    ctx = ExitStack()

    # ---------------- constants ----------------
    const = ctx.enter_context(tc.tile_pool(name="const", bufs=1))
    ident = const.tile([P, P], F32R)
    scratch = const.tile([P, P], F32, tag="idscratch")
    make_identity(nc, scratch)
    nc.vector.tensor_copy(ident, scratch)

    # causal mask helper: wm[p, f] = 1.0 iff f - p - 384 >= 0
    wm = const.tile([P, 896], BF16, tag="wm")
    nc.gpsimd.memset(wm, 1.0)
    nc.gpsimd.affine_select(
        out=wm,
        in_=wm,
        compare_op=mybir.AluOpType.is_ge,
        fill=0.0,
        base=-384,
        channel_multiplier=-1,
        pattern=[[1, 896]],
    )

    # ---------------- persistent tiles ----------------
    big = ctx.enter_context(tc.tile_pool(name="big", bufs=1))
    xT = big.tile([P, 8, T], F32R)              # [d%128, d//128, t]    64KB
    QTz = big.tile([P, H_LOC, T], BF16)         # zero-padded Q per head 32KB
    KT = big.tile([P, 4, T], BF16)              # [2-head pack, sub, t] 16KB
    V_aug = big.tile([P, 16, H_LOC, HD + 1], BF16)  # [j%128, jb, h, dd|1]
    AT = big.tile([P, 4, T], BF16)              # attention out         16KB
    nc.gpsimd.memset(V_aug[:, :, :, HD], 1.0)   # ones col -> rowsums via AV
    # zero the never-written half of each head's padded Q (once, on Pool)
    for h2 in range(H_LOC):
        z0 = 64 if h2 % 2 == 0 else 0
        nc.gpsimd.memset(QTz[z0:z0 + 64, h2, :], 0.0)

    xa = x_ap.rearrange("(tb p) d -> tb p d", p=P)            # [16, 128, 1024]
    wall = wqkv_ap.rearrange("(o p) c -> p o c", p=P)         # [128, 8, 1536]

    NTB = T // P

    # ---- phase A: x -> xT (transpose), V-proj software-pipelined ----
    with tc.tile_pool(name="lda", bufs=2) as lda, \
         tc.tile_pool(name="ldw", bufs=1) as ldw, \
         tc.tile_pool(name="psA", bufs=2, space=PSUM) as psA:
        # DMA emission order sets per-queue FIFO order: first x tiles, then
        # V weights (needed ~10us in), remaining x, then QK weights (needed
        # only when phase B starts).
        pre = {}

        def xload(tb):
            xc = lda.tile([P, D], F32R, tag="xin")
            nc.sync.dma_start(xc, xa[tb].bitcast(F32R))
            return xc

        for tb in range(4):
            pre[tb] = xload(tb)
        wv_sb = ldw.tile([P, 8, CLOC], F32R, tag="wv")
        nc.sync.dma_start(wv_sb, wall[:, :, 2 * CLOC:3 * CLOC].bitcast(F32R))
        w_sb = ldw.tile([P, 8, 2 * CLOC], F32R, tag="wqk")
        nc.sync.dma_start(w_sb, wall[:, :, 0:2 * CLOC].bitcast(F32R))

        def vproj(tb):
            psv = psA.tile([P, CLOC], F32, tag="ps_v")
            for k in range(8):
                nc.tensor.matmul(
                    psv,
                    xT[:, k, tb * P:(tb + 1) * P],
                    wv_sb[:, k, :],
                    start=(k == 0),
                    stop=(k == 7),
                )
            nc.vector.tensor_copy(
                V_aug[:, tb, :, 0:HD],
                psv.rearrange("p (h d) -> p h d", h=H_LOC),
            )

        for tb in range(NTB):
            xc = pre.pop(tb) if tb in pre else xload(tb)
            for half in range(2):
                pt = psA.tile([P, 512], F32R, tag="ps_t")
                for q in range(4):
                    db = half * 4 + q
                    nc.tensor.transpose(
                        pt[:, q * P:(q + 1) * P],
                        xc[:, db * P:(db + 1) * P],
                        ident,
                    )
                nc.vector.tensor_copy(
                    xT[:, half * 4:(half + 1) * 4, tb * P:(tb + 1) * P],
                    pt.rearrange("p (a b) -> p a b", a=4),
                )
            if tb >= 1:
                vproj(tb - 1)
        vproj(NTB - 1)

        # ---- phase B: Q^T / K^T proj -> QTz (padded) / KT ----
        with tc.tile_pool(name="psB", bufs=2, space=PSUM) as psB:
            for cb in range(8):
                sub = cb % 4
                for it in range(4):
                    ps = psB.tile([P, 512], F32, tag="ps_qkv")
                    for k in range(8):
                        nc.tensor.matmul(
                            ps,
                            w_sb[:, k, cb * P:(cb + 1) * P],
                            xT[:, k, it * 512:(it + 1) * 512],
                            start=(k == 0),
                            stop=(k == 7),
                        )
                    cols = slice(it * 512, (it + 1) * 512)
                    if cb < 4:
                        nc.vector.tensor_copy(
                            QTz[0:64, 2 * sub, cols], ps[0:64, :]
                        )
                        nc.vector.tensor_copy(
                            QTz[64:128, 2 * sub + 1, cols], ps[64:128, :]
                        )
                    else:
                        nc.vector.tensor_copy(KT[:, sub, cols], ps)

    # preload + cast w_out during attention so phase D pieces can start
    ldo = ctx.enter_context(tc.tile_pool(name="ldo", bufs=1))
    wo_st = ldo.tile([P, 4, D], F32, tag="wo_st")
    nc.sync.dma_start(wo_st, wout_ap.rearrange("(o p) n -> p o n", p=P))
    wo_sb = ldo.tile([P, 4, D], BF16, tag="wo_sb")
    nc.vector.tensor_copy(wo_sb, wo_st)

    oa = out_ap.rearrange("(tb p) d -> tb p d", p=P)

    # -------- phase C (+ interleaved D pieces): causal attention --------
    with tc.tile_pool(name="att", bufs=6) as att_pool, \
         tc.tile_pool(name="sm", bufs=2) as sm_pool, \
         tc.tile_pool(name="ypool", bufs=3) as ypool, \
         tc.tile_pool(name="psS", bufs=2, space=PSUM) as psS, \
         tc.tile_pool(name="psO", bufs=2, space=PSUM) as psO, \
         tc.tile_pool(name="psD", bufs=2, space=PSUM) as psD:

        def emit_sc(it, h, jb2):
            """Scores pair -> exp -> causal mask; returns the es tile."""
            i0 = it * 512
            ps = psS.tile([P, 1024], F32, tag="ps_s")
            for u in range(2):
                jb = 2 * jb2 + u
                nc.tensor.matmul(
                    ps[:, u * 512:(u + 1) * 512],
                    KT[:, h // 2, jb * P:(jb + 1) * P],
                    QTz[:, h, i0:i0 + 512],
                    start=True,
                    stop=True,
                )
            es = att_pool.tile([P, 1024], BF16, tag="es")
            nc.scalar.activation(es, ps, Exp, scale=0.125)
            for u in range(2):
                jb = 2 * jb2 + u
                off = jb * P - i0
                if off >= 0:  # diagonal tile: zero out j > i
                    s = 384 - off
                    nc.vector.tensor_tensor(
                        es[:, u * 512:(u + 1) * 512],
                        es[:, u * 512:(u + 1) * 512],
                        wm[:, s:s + 512],
                        mult,
                    )
            return es

        def emit_av(it, h, jb2, es, po):
            last = 2 * (it + 1) - 1
            for u in range(2):
                jb = 2 * jb2 + u
                nc.tensor.matmul(
                    po[0:HD + 1, :],
                    V_aug[:, jb, h, :],
                    es[:, u * 512:(u + 1) * 512],
                    start=(jb2 == 0 and u == 0),
                    stop=(jb2 == last and u == 1),
                )

        def emit_norm(it, h, po):
            i0 = it * 512
            row0 = (h % 2) * 64
            sub = h // 2
            rr = sm_pool.tile([1, 512], F32, tag="rr")
            nc.vector.tensor_copy(rr, po[HD:HD + 1, :])
            rf = sm_pool.tile([1, 512], F32, tag="rf")
            nc.vector.reciprocal_approx_fast(out=rf, in_=rr)
            rb = sm_pool.tile([64, 512], F32, tag="rb")
            nc.gpsimd.partition_broadcast(rb, rf)
            nc.vector.tensor_tensor(
                AT[row0:row0 + 64, sub, i0:i0 + 512],
                po[0:64, :],
                rb,
                mult,
            )

        def emit_d_piece(tb, nt):
            py = psD.tile([P, 512], F32, tag="ps_y")
            for k in range(4):
                nc.tensor.matmul(
                    py,
                    AT[:, k, tb * P:(tb + 1) * P],
                    wo_sb[:, k, nt * 512:(nt + 1) * 512],
                    start=(k == 0),
                    stop=(k == 3),
                )
            ysb = ypool.tile([P, 512], BF16, tag="ysb")
            nc.vector.tensor_copy(ysb, py)
            nc.sync.dma_start(oa[tb, :, nt * 512:(nt + 1) * 512], ysb)

        tiles = [
            (it, h, jb2)
            for it in range(4)
            for h in range(H_LOC)
            for jb2 in range(2 * (it + 1))
        ]
        d_queue = []       # D pieces ready to interleave
        po_live = {}
        pend = None        # (it, h, jb2, es) awaiting AV

        def drain_pend():
            nonlocal pend
            if pend is None:
                return
            pit, ph, pjb2, pes = pend
            pend = None
            emit_av(pit, ph, pjb2, pes, po_live[(pit, ph)])
            if pjb2 == 2 * (pit + 1) - 1:
                emit_norm(pit, ph, po_live.pop((pit, ph)))
                if ph == H_LOC - 1:
                    # epoch done: its 4 token blocks become D pieces
                    for tb in range(4 * pit, 4 * pit + 4):
                        for nt in range(2):
                            d_queue.append((tb, nt))
                elif d_queue:
                    emit_d_piece(*d_queue.pop(0))

        for (it, h, jb2) in tiles:
            if jb2 == 0:
                po_new = psO.tile([P, 512], F32, tag="ps_o")
                po_live[(it, h)] = po_new
            es = emit_sc(it, h, jb2)
            drain_pend()
            pend = (it, h, jb2, es)
        drain_pend()
        for piece in d_queue:
            emit_d_piece(*piece)

    ctx.close()


_CACHE = {}


def _get_nc(mode=None):
    key = "mixed"
    if key in _CACHE:
        return _CACHE[key]
    nc = bacc.Bacc(
        "TRN2",
        target_bir_lowering=False,
        debug=False,
        enable_asserts=False,
        num_devices=N_CORES,
    )
    x_d = nc.dram_tensor("x", [T, D], F32, kind="ExternalInput")
    wqkv_d = nc.dram_tensor("w_qkv", [D, 3 * CLOC], F32, kind="ExternalInput")
    wout_d = nc.dram_tensor("w_out", [CLOC, D], F32, kind="ExternalInput")
    out_d = nc.dram_tensor("out", [T, D], BF16, kind="ExternalOutput")
    with tile.TileContext(nc) as tc:
        _build_kernel_body(
            nc, tc, x_d.ap(), wqkv_d.ap(), wout_d.ap(), out_d.ap()
        )
    nc.compile()
    _CACHE[key] = nc
    return nc


def _make_in_maps(x, w_qkv, w_out):
    x = np.ascontiguousarray(np.asarray(x, dtype=np.float32))
    w_qkv = np.ascontiguousarray(np.asarray(w_qkv, dtype=np.float32))
    w_out = np.ascontiguousarray(np.asarray(w_out, dtype=np.float32))
    in_maps = []
    for c in range(N_CORES):
        b, g = divmod(c, 2)
        c0 = g * CLOC
        wloc = np.concatenate(
            [
                w_qkv[:, c0:c0 + CLOC],
                w_qkv[:, D + c0:D + c0 + CLOC],
                w_qkv[:, 2 * D + c0:2 * D + c0 + CLOC],
            ],
            axis=1,
        )
        in_maps.append({
            "x": np.ascontiguousarray(x[b]),
            "w_qkv": np.ascontiguousarray(wloc),
            "w_out": np.ascontiguousarray(w_out[c0:c0 + CLOC]),
        })
    return in_maps


def run(x, w_qkv, w_out, trace=False, mode=None):
    nc = _get_nc(mode)
    in_maps = _make_in_maps(x, w_qkv, w_out)
    res = bass_utils.run_bass_kernel_spmd(
        nc, in_maps, core_ids=list(range(N_CORES)), trace=trace
    )
    y = np.empty((B, T, D), dtype=np.float32)
    for b in range(B):
        y[b] = (
            np.asarray(res.results[2 * b]["out"], dtype=np.float32)
            + np.asarray(res.results[2 * b + 1]["out"], dtype=np.float32)
        )
    return y, res


def kernel(x, w_qkv, w_out):
    y, _ = run(x, w_qkv, w_out, trace=False)
    return y
